# revision 33
# baseline (speedup 1.0000x reference)
"""DecoderTreeRNN Trainium2 kernel.

Computes: h0 = relu(encoding); expand a depth-`depth` binary tree with two
zero-input GRU cells (left/right); project every leaf hidden state with W_out
and take log_softmax over the vocab.

Strategy: pure data parallel over 8 NeuronCores (batch sharded), GRU weights
and the output projection replicated.  On-core layout is transposed
([hidden-chunk on partitions, tokens on the free dim]) so all matmuls
contract over partitions and the softmax reduction runs along the free dim.

v2: the output projection runs in fp8 (DoubleRow matmuls, 2x tensor rate),
y and the stored output are bf16 (host upcasts to fp32; log-probs ~-9 so
bf16 keeps elementwise rel err ~2e-3, well under the 2e-2 gate), and the
log_softmax tail subtract is split DVE/ACT per segment.

v3: engine rebalance in the projection.  The v2 chunk cadence (17.1us) was
set by ACT (5 exps + 2 Identity-subtract segments + 5 accum reads per
chunk).  Now ACT runs just TWO merged exps per chunk plus one Ln that sums
the two accumulators via its per-partition bias operand (ln(s0+s1)); the
log_softmax subtract moves to GPSIMD (4 segments, previously idle) + DVE
(1 segment at 4x).  The cadence floor becomes the DVE PSUM drain (fp32
PSUM reads are fixed at 1 elem/cycle/lane), ~11.5us/chunk.
"""

import os
import sys
from contextlib import ExitStack

import numpy as np

for _p in ("/opt/trn_rl_repo", "/root/.axon_site/_ro/trn_rl_repo"):
    if os.path.isdir(_p) and _p not in sys.path:
        sys.path.insert(0, _p)

import ml_dtypes

N_CORES = 8
P = 128
TTILE = 512  # token tile for GRU matmuls (max fp32 moving free dim)
NBF = 512  # fp32 elements per PSUM bank
VGW = 4 * NBF  # vocab group width (4 PSUM banks; 2 rotating slots)

# Set by test harness to capture a profile on the next kernel() call.
TRACE = False
LAST_EXEC_NS = None
LAST_RESULTS = None

_COMPILE_CACHE = {}


def _bitrev(x, bits):
    r = 0
    for _ in range(bits):
        r = (r << 1) | (x & 1)
        x >>= 1
    return r


def _numpy_reference(encoding, W_hh_l, b_ih_l, b_hh_l, W_hh_r, b_ih_r, b_hh_r,
                     W_out, b_out, depth):
    def gru(h, W, b_ih, b_hh):
        Hd = h.shape[-1]
        gh = h @ W.T + b_hh
        r = 1.0 / (1.0 + np.exp(-(b_ih[:Hd] + gh[..., :Hd])))
        z = 1.0 / (1.0 + np.exp(-(b_ih[Hd:2 * Hd] + gh[..., Hd:2 * Hd])))
        n = np.tanh(b_ih[2 * Hd:] + r * gh[..., 2 * Hd:])
        return (1.0 - z) * n + z * h

    h = np.maximum(encoding, 0.0)[:, None, :]
    for _ in range(depth):
        left = gru(h, W_hh_l, b_ih_l, b_hh_l)
        right = gru(h, W_hh_r, b_ih_r, b_hh_r)
        h = np.stack([left, right], axis=2).reshape(h.shape[0], -1, h.shape[-1])
    logits = h @ W_out.T + b_out
    m = logits.max(axis=-1, keepdims=True)
    e = np.exp(logits - m)
    return (logits - m) - np.log(e.sum(axis=-1, keepdims=True))


def _patch_act_tables(bacc, mybir):
    """Constrain the ACT table-set chooser so the GRU phase and the
    projection phase each stick to ONE set (2 loads total instead of 2
    per token chunk).  Only the chooser's view is filtered; the runtime
    tables are the real (full) sets, so execution is unchanged."""
    from concourse import hw_specs
    AF = mybir.ActivationFunctionType
    orig = hw_specs.get_activation_tables
    if getattr(bacc.get_activation_tables, "_treernn_patch", False):
        return
    keep = {
        "sigmoid_and_others": {AF.Sigmoid, AF.Tanh, AF.Relu},
        "natural_log_exp_and_others": {AF.Exp, AF.Ln, AF.Identity, AF.Copy},
    }
    controlled = set().union(*keep.values())

    def patched(arch):
        tabs = {k: set(v) for k, v in orig(arch).items()}
        for name, s in tabs.items():
            s -= controlled
            s |= keep.get(name, set())
        return tabs

    patched._treernn_patch = True
    bacc.get_activation_tables = patched


def _build(Bc, H, V, depth):
    """Build + compile the single-core SPMD program (identical on all cores)."""
    import concourse.bass as bass  # noqa: F401
    import concourse.tile as tile
    from concourse import bacc, mybir

    f32 = mybir.dt.float32
    bf16 = mybir.dt.bfloat16
    f8 = mybir.dt.float8e4
    AF = mybir.ActivationFunctionType
    OP = mybir.AluOpType
    DR = mybir.MatmulPerfMode.DoubleRow
    _patch_act_tables(bacc, mybir)

    KH = H // P
    H3 = 3 * H
    L = 1 << depth
    TOK = Bc * L
    NTC = (TOK + P - 1) // P

    def _chunks(width):
        out, pos = [], 0
        while pos < V:
            w = min(width, V - pos)
            out.append((pos, w))
            pos += w
        return out

    vgroups = _chunks(VGW)   # PSUM drain / final-subtract granularity
    NVG = len(vgroups)
    assert NVG >= 3
    # two exp groups per chunk: [0, EXP_SPLIT) after the third drain,
    # [EXP_SPLIT, V) after the last
    EXP_SPLIT = vgroups[2][0] + vgroups[2][1]

    nc = bacc.Bacc("TRN2", target_bir_lowering=False, debug=False,
                   num_devices=N_CORES)

    enc_d = nc.dram_tensor("enc_t", [P, KH, Bc], f32, kind="ExternalInput").ap()
    whh_d = {s: nc.dram_tensor(f"whht_{s}", [P, KH, H3], bf16,
                               kind="ExternalInput").ap() for s in "lr"}
    whh8_d = {s: nc.dram_tensor(f"whht8_{s}", [P, KH, H3], f8,
                                kind="ExternalInput").ap() for s in "lr"}
    # packed per-side biases: cols [0:2K]=sigmoid(r,z), [2K:3K]=tanh, [3K:4K]=n_hh
    bias_d = {s: nc.dram_tensor(f"bias_{s}", [P, 4 * KH], f32,
                                kind="ExternalInput").ap() for s in "lr"}
    wout_d = nc.dram_tensor("woutt", [P, KH, V], f8, kind="ExternalInput").ap()
    bout_d = nc.dram_tensor("bout", [P, V], bf16, kind="ExternalInput").ap()
    out_d = nc.dram_tensor("out", [Bc, L, V], bf16, kind="ExternalOutput").ap()

    with tile.TileContext(nc) as tc, ExitStack() as ctx:
        constp = ctx.enter_context(tc.tile_pool(name="const", bufs=1))
        ht2p = ctx.enter_context(tc.tile_pool(name="ht2", bufs=1))
        ht2 = ht2p.tile([P, KH, TOK], f8)
        wvep = ctx.enter_context(tc.tile_pool(name="wout_early", bufs=1))
        bop = ctx.enter_context(tc.tile_pool(name="bout", bufs=1))

        bsig, btanh, bnhh = {}, {}, {}
        for s in "lr":
            bt = constp.tile([P, 4 * KH], f32, name=f"bias{s}")
            nc.sync.dma_start(out=bt, in_=bias_d[s])
            bsig[s] = bt[:, :2 * KH]
            btanh[s] = bt[:, 2 * KH:3 * KH]
            bnhh[s] = bt[:, 3 * KH:]
        # warm the sigmoid/tanh ACT table set while the weight DMAs run
        # (first real sigmoid would otherwise eat the ~2.7us table load)
        warm = constp.tile([P, 1], f32, name="warm")
        nc.scalar.activation(out=warm, in_=bsig["l"][:, 0:1], func=AF.Sigmoid)

        def load_proj_consts():
            # issued AFTER the GRU weights so they don't head-block the
            # sync-engine DMA queue and delay the GRU start
            bout_sb = bop.tile([P, V], bf16)
            nc.sync.dma_start(out=bout_sb, in_=bout_d)
            wv = []
            for vg, (vs, vw) in enumerate(vgroups):
                wt = wvep.tile([P, KH, vw], f8, name=f"wv{vg}")
                nc.sync.dma_start(out=wt, in_=wout_d[:, :, vs:vs + vw])
                wv.append(wt)
            return bout_sb, wv

        # ---------------- GRU tree expansion ----------------
        NLL = max(1, min(P // Bc, L))  # leaves (nodes) per token chunk
        DR_LVL = depth - 2  # levels >= this run fp8 DoubleRow matmuls
        with tc.tile_pool(name="gwhh", bufs=1) as gwp, \
             tc.tile_pool(name="gh", bufs=1) as ghp, \
             tc.tile_pool(name="gh8", bufs=1) as g8p, \
             tc.tile_pool(name="ghfin", bufs=2) as gfp, \
             tc.tile_pool(name="gact", bufs=2) as gap, \
             tc.tile_pool(name="gactd", bufs=1) as gdp, \
             tc.tile_pool(name="gpsum", bufs=2, space="PSUM") as gpp:
            # startup order: encoding (relu can start at once), then the GRU
            # weights the first level needs, then everything else
            enc_sb = gdp.tile([P, KH, Bc], f32, name="enc_stage")
            nc.sync.dma_start(out=enc_sb, in_=enc_d)
            whh = {}
            for s in "lr":
                whh[s] = gwp.tile([P, KH, H3], bf16, name=f"whh{s}")
            # per-gate weight DMAs so the first matmuls only wait for 1/6th
            # of the 3MB of GRU weights
            for s in "lr":
                for g0 in range(0, H3, H):
                    nc.sync.dma_start(out=whh[s][:, :, g0:g0 + H],
                                      in_=whh_d[s][:, :, g0:g0 + H])
            # fp8 weight copies for the DoubleRow levels (needed ~60us in)
            whh8 = {}
            for s in "lr":
                whh8[s] = gwp.tile([P, KH, H3], f8, name=f"whh8{s}")
                nc.sync.dma_start(out=whh8[s], in_=whh8_d[s])
            h_cur = ghp.tile([P, KH, Bc], bf16, name="h_l0")
            # relu on DVE keeps ACT's first table set = sigmoid's
            nc.vector.tensor_scalar(out=h_cur, in0=enc_sb, scalar1=0.0,
                                    scalar2=None, op0=OP.max)

            bout_sb, wv = load_proj_consts()

            def emit_copies(h_fin, si, t0, tt):
                """ht2 permute for the nodes finished by final-level tile
                (si, t0); node-grouped chunks: chunk tci holds nodes
                [NLL*tci, NLL*(tci+1)), interleaved p = b*NLL + (node % NLL).
                h_fin is the per-tile scratch (tokens local to the tile)."""
                t_half = Bc << (depth - 1)
                base_tok = si * t_half + t0
                lo = base_tok // Bc
                for node in range(lo, lo + tt // Bc):
                    base = (node // NLL) * P + node % NLL
                    lt = node * Bc - base_tok
                    nc.vector.tensor_copy(
                        out=ht2[:, :, base:base + (Bc - 1) * NLL + 1:NLL],
                        in_=h_fin[:, :, lt:lt + Bc])

            h8_cur = None
            for lvl in range(depth):
                t = Bc << lvl
                last = lvl == depth - 1
                use_dr = lvl >= DR_LVL and KH % 2 == 0 and h8_cur is not None
                make8 = not last and lvl + 1 >= DR_LVL and KH % 2 == 0
                h_nxt = None if last else ghp.tile([P, KH, 2 * t], bf16,
                                                   name=f"h_l{lvl + 1}")
                h8_nxt = g8p.tile([P, KH, 2 * t], f8,
                                  name=f"h8_l{lvl + 1}") if make8 else None
                for si, s in enumerate("lr"):
                    soff = si * t
                    for t0 in range(0, t, TTILE):
                        tt = min(TTILE, t - t0)
                        hs = h_cur[:, :, t0:t0 + tt]
                        r_sb = gap.tile([P, KH, TTILE], bf16, name="g_r")[:, :, :tt]
                        z_sb = gap.tile([P, KH, TTILE], bf16, name="g_z")[:, :, :tt]
                        n_sb = gap.tile([P, KH, TTILE], bf16, name="g_n")[:, :, :tt]
                        d_sb = gdp.tile([P, KH, TTILE], bf16, name="g_d")[:, :, :tt]
                        for gi in range(3):  # r, z, n
                            ps = gpp.tile([P, KH, TTILE], f32,
                                          name="g_ps")[:, :, :tt]
                            for gc in range(KH):
                                col = gi * H + gc * P
                                if use_dr:
                                    # fp8 DoubleRow: 2 contraction chunks
                                    # per instruction, 2x PE rate
                                    for kp in range(0, KH, 2):
                                        nc.tensor.matmul(
                                            ps[:, gc, :],
                                            lhsT=whh8[s][:, kp:kp + 2,
                                                         col:col + P],
                                            rhs=h8_cur[:, kp:kp + 2,
                                                       t0:t0 + tt],
                                            start=(kp == 0),
                                            stop=(kp == KH - 2),
                                            perf_mode=DR)
                                    continue
                                for k in range(KH):
                                    nc.tensor.matmul(
                                        ps[:, gc, :],
                                        lhsT=whh[s][:, k, col:col + P],
                                        rhs=hs[:, k, :],
                                        start=(k == 0), stop=(k == KH - 1))
                            if gi == 0:
                                for gc in range(KH):
                                    nc.scalar.activation(
                                        out=r_sb[:, gc, :], in_=ps[:, gc, :],
                                        func=AF.Sigmoid,
                                        bias=bsig[s][:, gc:gc + 1])
                            elif gi == 1:
                                for gc in range(KH):
                                    nc.scalar.activation(
                                        out=z_sb[:, gc, :], in_=ps[:, gc, :],
                                        func=AF.Sigmoid,
                                        bias=bsig[s][:, KH + gc:KH + gc + 1])
                            else:
                                for gc in range(KH):
                                    # n_pre = (gh_n + b_hh_n) * r
                                    nc.vector.scalar_tensor_tensor(
                                        out=n_sb[:, gc, :], in0=ps[:, gc, :],
                                        scalar=bnhh[s][:, gc:gc + 1],
                                        in1=r_sb[:, gc, :],
                                        op0=OP.add, op1=OP.mult)
                                for gc in range(KH):
                                    nc.scalar.activation(
                                        out=n_sb[:, gc, :], in_=n_sb[:, gc, :],
                                        func=AF.Tanh,
                                        bias=btanh[s][:, gc:gc + 1])
                        # h' = n + z * (h - n)
                        nc.vector.tensor_tensor(d_sb, hs, n_sb, OP.subtract)
                        nc.vector.tensor_tensor(d_sb, d_sb, z_sb, OP.mult)
                        if last:
                            # the final level's bf16 output is only read by
                            # the ht2 staging copies: write a per-tile
                            # scratch instead of a [P,KH,2048] tensor
                            h_fin = gfp.tile([P, KH, TTILE], bf16,
                                             name="h_fin")[:, :, :tt]
                            nc.vector.tensor_tensor(h_fin, d_sb, n_sb,
                                                    OP.add)
                            emit_copies(h_fin, si, t0, tt)
                        else:
                            dst = h_nxt[:, :, soff + t0:soff + t0 + tt]
                            nc.vector.tensor_tensor(dst, d_sb, n_sb, OP.add)
                            if make8:
                                nc.vector.tensor_copy(
                                    out=h8_nxt[:, :, soff + t0:soff + t0 + tt],
                                    in_=dst)
                h_cur, h8_cur = h_nxt, h8_nxt

        # swap in the exp/ln ACT table set now, overlapping chunk 0's
        # matmuls + drains instead of stalling chunk 0's first exp
        nc.scalar.activation(out=warm, in_=warm, func=AF.Exp)

        # ---------------- output projection + log_softmax ----------------
        # y bufs=4: each out-DMA takes ~10us on its ring, so DMA completions
        # lag ~2 chunks behind issue; at bufs=3 the Tile WAR barriers
        # (drain(k+1) vs DMAs(k-2)) bind and gridlock the DVE queue
        with tc.tile_pool(name="ypool", bufs=4) as yp, \
             tc.tile_pool(name="stat", bufs=8) as stp, \
             tc.tile_pool(name="escratch", bufs=2) as esp, \
             tc.tile_pool(name="ppsum", bufs=2, space="PSUM") as ppp:

            def out_dma(tci, pc, y, v0, v1):
                """Per-leaf stores; partitions are interleaved (p = b*NLL+ll)
                so each DMA's 32 source partitions stride across all 16 SBUF
                port groups and its rows spread over all 16 SDMA engines.
                Chunk tci holds NODES [NLL*tci, NLL*(tci+1)); the in-order
                leaf index is the bit-reversed node id."""
                nll = pc // Bc
                for ll in range(nll):
                    leaf = _bitrev(tci * nll + ll, depth)
                    nc.sync.dma_start(out=out_d[:, leaf, v0:v1],
                                      in_=y[ll:pc:nll, v0:v1])

            def sub_seg(eng, y, cs, a, b):
                """y[a:b] -= c on the chosen engine (cs[:,0:1]=c, [:,1:2]=-c).
                GPSIMD is NOT an option: its tensor_scalar ucode measures
                ~17 cyc/elem AND it contends with DVE for the shared POOL
                SBUF port, wrecking the drains."""
                if eng == "A":
                    nc.scalar.activation(out=y[:, a:b], in_=y[:, a:b],
                                         func=AF.Identity, bias=cs[:, 1:2])
                else:
                    nc.vector.tensor_scalar(out=y[:, a:b], in0=y[:, a:b],
                                            scalar1=cs[:, 0:1], scalar2=None,
                                            op0=OP.subtract)

            def tail_head(st):
                """c = ln(s0+s1+s2+s3) ENTIRELY on ACT (Identity per-
                partition-bias adds + Ln): any DVE op here would be queued
                at the chunk boundary where its wait on the accumulator
                reads blocks the drain queue ~4us."""
                tci, pc, y, sums, cs = st
                nc.scalar.activation(out=cs[:, 2:3], in_=sums[:, 0:1],
                                     func=AF.Identity, bias=sums[:, 1:2])
                nc.scalar.activation(out=cs[:, 3:4], in_=sums[:, 2:3],
                                     func=AF.Identity, bias=sums[:, 3:4])
                nc.scalar.activation(out=cs[:, 0:1], in_=cs[:, 2:3],
                                     func=AF.Ln, bias=cs[:, 3:4])
                nc.scalar.activation(out=cs[:, 1:2], in_=cs[:, 0:1],
                                     func=AF.Identity, scale=-1.0)

            def tails(st, y_cur):
                """All of chunk p0's subtracts + DMAs, issued right after
                the CURRENT chunk's final drain.  The 1-wide fence reads
                y_cur[V-1] (just written by that drain, a clean RAW dep) so
                the scheduler cannot hoist this block ahead of the drains -
                that head-of-line blocking was worth ~5us/chunk."""
                tci, pc, y, sums, cs = st
                fence = stp.tile([P, 1], f32, name="fence")[:pc]
                nc.vector.scalar_tensor_tensor(
                    out=fence, in0=y_cur[:, V - 1:V], scalar=0.0,
                    in1=cs[:, 0:1], op0=OP.mult, op1=OP.add)
                for s, eng in ((0, "V"), (1, "V"), (2, "A"), (3, "V"),
                               (4, "V")):
                    vs, vw = vgroups[s]
                    if eng == "A":
                        nc.scalar.activation(out=y[:, vs:vs + vw],
                                             in_=y[:, vs:vs + vw],
                                             func=AF.Identity,
                                             bias=cs[:, 1:2])
                    else:
                        nc.vector.tensor_scalar(out=y[:, vs:vs + vw],
                                                in0=y[:, vs:vs + vw],
                                                scalar1=fence, scalar2=None,
                                                op0=OP.subtract)
                    if s == 1:
                        out_dma(tci, pc, y, 0, vgroups[2][0])
                    elif s == 3:
                        out_dma(tci, pc, y, vgroups[2][0], vgroups[4][0])
                    elif s == 4:
                        out_dma(tci, pc, y, vgroups[4][0], V)

            # exp groups: [0:2048) [2048:4096) [4096:8192) [8192:10000).
            # Merging the middle pair saves ~1.7us/chunk of ACT instruction
            # + accumulator-read overhead; the LAST group stays small so the
            # Ln is ready ~2.4us after the final drain - chunk k's tail work
            # (issued during chunks k+1/k+2, baseline-style) then never
            # stalls the DVE queue.
            p0 = p1 = None
            for tci in range(NTC):
                pc = min(P, TOK - tci * P)  # tokens in this chunk
                y = yp.tile([P, V], bf16, name="y")[:pc]
                sums = stp.tile([P, 4], f32, name="sums")[:pc]
                cs = stp.tile([P, 4], f32, name="cs")[:pc]
                for vg, (vs, vw) in enumerate(vgroups):
                    ps = ppp.tile([P, VGW], f32, name="p_vg")[:pc, :vw]
                    for vt0 in range(0, vw, NBF):
                        w = min(NBF, vw - vt0)
                        pslice = ps[:, vt0:vt0 + w]
                        for kp in range(0, KH, 2):
                            nc.tensor.matmul(
                                pslice,
                                lhsT=ht2[:, kp:kp + 2, tci * P:tci * P + pc],
                                rhs=wv[vg][:, kp:kp + 2, vt0:vt0 + w],
                                start=(kp == 0), stop=(kp == KH - 2),
                                perf_mode=DR)
                    # PSUM drain + bias add -> y (bf16); single PSUM read
                    nc.vector.tensor_tensor(
                        y[:, vs:vs + vw], ps, bout_sb[:pc, vs:vs + vw], OP.add)
                    if vg == 0:
                        esc = esp.tile([P, 4096], bf16, name="e")[:pc, :2048]
                        nc.scalar.activation(out=esc, in_=y[:, :2048],
                                             func=AF.Exp,
                                             accum_out=sums[:, 0:1])
                    elif vg == 1:
                        # previous chunk's Ln chain (pure ACT) right after
                        # exp G0 so it's long done when its subs run
                        if p1 is not None:
                            tail_head(p1)
                        esc = esp.tile([P, 4096], bf16, name="e")[:pc, :2048]
                        nc.scalar.activation(out=esc, in_=y[:, 2048:4096],
                                             func=AF.Exp,
                                             accum_out=sums[:, 1:2])
                    elif vg == 3:
                        esc = esp.tile([P, 4096], bf16, name="e")[:pc]
                        nc.scalar.activation(out=esc, in_=y[:, 4096:8192],
                                             func=AF.Exp,
                                             accum_out=sums[:, 2:3])
                    elif vg == 4:
                        esc = esp.tile([P, 4096], bf16,
                                       name="e")[:pc, :V - 8192]
                        nc.scalar.activation(out=esc, in_=y[:, 8192:V],
                                             func=AF.Exp,
                                             accum_out=sums[:, 3:4])
                        if p0 is not None:
                            tails(p0, y)
                p0, p1 = p1, (tci, pc, y, sums, cs)
            # epilogue: finish the two still-pending tails
            tail_head(p1)
            tails(p0, p0[2])
            tci, pc, y, _, cs = p1
            esegs = [("V", 0, 2048), ("A", 2048, 4096), ("V", 4096, 6144),
                     ("A", 6144, 8192), ("V", 8192, V)]
            for eng, a, b in esegs:
                sub_seg(eng, y, cs, a, b)
                out_dma(tci, pc, y, a, b)

    nc.compile()
    return nc


def _packed_bias(b_ih, b_hh, H, KH):
    """[P, 4*KH]: sigmoid biases (b_ih+b_hh for r,z), tanh bias (b_ih_n),
    and the pre-multiply n-gate bias (b_hh_n), per 128-row chunk."""
    P = 128
    sig = (b_ih + b_hh)[:2 * H].reshape(2 * KH, P).T
    tanh = b_ih[2 * H:].reshape(KH, P).T
    nhh = b_hh[2 * H:].reshape(KH, P).T
    return np.ascontiguousarray(np.concatenate([sig, tanh, nhh], axis=1))


def _get_compiled(Bc, H, V, depth):
    key = (Bc, H, V, depth)
    if key not in _COMPILE_CACHE:
        _COMPILE_CACHE[key] = _build(Bc, H, V, depth)
    return _COMPILE_CACHE[key]


def kernel(encoding, W_hh_l, b_ih_l, b_hh_l, W_hh_r, b_ih_r, b_hh_r,
           W_out, b_out, depth):
    global LAST_EXEC_NS, LAST_RESULTS
    encoding = np.asarray(encoding, np.float32)
    W_hh = {"l": np.asarray(W_hh_l, np.float32), "r": np.asarray(W_hh_r, np.float32)}
    b_ih = {"l": np.asarray(b_ih_l, np.float32), "r": np.asarray(b_ih_r, np.float32)}
    b_hh = {"l": np.asarray(b_hh_l, np.float32), "r": np.asarray(b_hh_r, np.float32)}
    W_out = np.asarray(W_out, np.float32)
    b_out = np.asarray(b_out, np.float32)
    depth = int(depth)

    B, H = encoding.shape
    V = W_out.shape[0]
    tok = (B // N_CORES) * (1 << depth) if B % N_CORES == 0 else 0
    if (depth < 1 or B % N_CORES or H % P or P % (B // N_CORES)
            or (tok % P != 0 and tok > P)):
        return _numpy_reference(encoding, W_hh["l"], b_ih["l"], b_hh["l"],
                                W_hh["r"], b_ih["r"], b_hh["r"],
                                W_out, b_out, depth).astype(np.float32)

    Bc = B // N_CORES
    KH = H // P
    bf16 = ml_dtypes.bfloat16
    f8 = ml_dtypes.float8_e4m3

    nc = _get_compiled(Bc, H, V, depth)

    # device layouts are [P(partition), KH, x]: H index = k*P + p -> axes (p, k)
    woutt = np.ascontiguousarray(
        W_out.T.astype(f8).reshape(KH, P, V).transpose(1, 0, 2))
    bout_b = np.ascontiguousarray(
        np.broadcast_to(b_out.astype(bf16)[None, :], (P, V)))
    shared = {"woutt": woutt, "bout": bout_b}
    for s in "lr":
        whht = W_hh[s].T.reshape(KH, P, 3 * H).transpose(1, 0, 2)
        shared[f"whht_{s}"] = np.ascontiguousarray(whht.astype(bf16))
        shared[f"whht8_{s}"] = np.ascontiguousarray(whht.astype(f8))
        shared[f"bias_{s}"] = _packed_bias(b_ih[s], b_hh[s], H, KH)

    encT = encoding.T  # [H, B]
    in_maps = []
    for c in range(N_CORES):
        enc_c = np.ascontiguousarray(
            encT[:, c * Bc:(c + 1) * Bc].reshape(KH, P, Bc).transpose(1, 0, 2))
        in_maps.append({"enc_t": enc_c, **shared})

    from concourse import bass_utils
    kw = {}
    if TRACE:
        kw["tmpdir"] = os.environ.get("BASS_TRACE_DIR") or None
    res = bass_utils.run_bass_kernel_spmd(
        nc, in_maps, core_ids=list(range(N_CORES)), trace=TRACE, **kw)
    LAST_EXEC_NS = res.exec_time_ns
    LAST_RESULTS = res
    out = np.concatenate([r["out"] for r in res.results], axis=0)
    return np.ascontiguousarray(out.astype(np.float32))

